# revision 43
# baseline (speedup 1.0000x reference)
"""Causal self-attention + cross-attention Trainium2 kernel (8 NeuronCores).

Sharding: head-parallel. 16 heads x 2 batches = 32 (b,h) pairs; core c owns
heads {2c, 2c+1} for both batches (its 128 channels of C=1024). Projections
are column-sliced per core; attention runs fully local per head; the output
projection is row-sliced and the 8 partial [NT, C] fp32 outputs are summed
on the host (no device collectives).

v3 (fp16 everywhere; fp8 was tried and rejected — softmax reweighting
noise from quantized Q/K lands ~5% relative error on y, over the 2e-2
gate): same software-pipelined emission as v2, with schedule fixes found
from NTFF traces:
- x chunks stream through a rotating 4-buffer SBUF pool; input DMAs are
  spread across the sync/scalar/gpsimd queues (each queue sustains only
  ~120 GB/s) with the first-needed tensors on their own queues, and
  cross-attention inputs split per batch so no consumer waits on a late
  whole-tile write.
- fillers are pumped BEFORE each AV flush (the PE queue is in-order; a
  flush waiting on exp would block fillers emitted after it).
- the softmax epilogue runs recips first (they gate the broadcast), reads
  yh head-b rows on the scalar engine (drained of exps at part end), puts
  the ya path on vector and yb on gpsimd in parallel, and writes head-b
  rows straight into yT2[64:128] (no SBUF shift DMA).
- out-projection items are spread across the following part as boundary
  filler; their output DMAs are 2-token-tile batched on the SWDGE queue.
- the V pad columns hold the ones-value, so the AV matmul replicates
  ALPHA*den across PSUM partitions 0:64 for free; the reciprocal runs on
  [64,512] at the same DVE cost as [1,512] (cost = free size), removing
  the per-part DRAM-roundtrip partition broadcast entirely.
- the final chunk's out-projection uses the freed score PSUM banks with
  vector+scalar evictions, and its output DMAs go on the idle sync queue.
"""
import sys

sys.path.insert(0, "/opt/trn_rl_repo")

import numpy as np
import ml_dtypes

import concourse.bass as bass
import concourse.tile as tile
from concourse import bacc, mybir
from concourse.bass_utils import run_bass_kernel_spmd

dt = mybir.dt

B, T, TC, C, CC, H, D = 2, 2048, 512, 1024, 512, 16, 64
NCORES = 8
CPC = 128          # channels per core = 2 heads * 64
NT = B * T         # 4096 tokens (batch-major)
NTC = B * TC       # 1024 cross tokens
KT_X = C // 128    # 8 contraction tiles over C
KT_C = CC // 128   # 4 contraction tiles over CC
NCH = NT // 512    # 8 token chunks (b0: 0-3, b1: 4-7)
NCHC = NTC // 512  # 2 cross chunks (b0, b1)
QC_PER_B = T // 512
KT_PER_B = T // 128
VW = 128           # V block cols per head: [ones | pad63 | v64]
ALPHA = 1.0 / 64   # ones-column value; denominators come out scaled by ALPHA


def _build(zero_bias=False):
    f32, f16, f8 = dt.float32, dt.float16, dt.float8e4
    nc = bacc.Bacc("TRN2", target_bir_lowering=False, debug=False,
                   enable_asserts=True, num_devices=NCORES)

    xTd16 = nc.dram_tensor("xT16", [NCH, 128, KT_X, 512], f16, kind="ExternalInput").ap()
    cTd16 = nc.dram_tensor("cT16", [NCHC, 128, KT_C, 512], f16, kind="ExternalInput").ap()
    wqd = nc.dram_tensor("wq", [128, KT_X, CPC], f16, kind="ExternalInput").ap()
    wkd = nc.dram_tensor("wk", [128, KT_X, CPC], f16, kind="ExternalInput").ap()
    wcqd = nc.dram_tensor("wcq", [128, KT_X, CPC], f16, kind="ExternalInput").ap()
    wckd = nc.dram_tensor("wck", [128, KT_C, CPC], f16, kind="ExternalInput").ap()
    wvd = nc.dram_tensor("wv", [128, KT_X, CPC], f16, kind="ExternalInput").ap()
    wcvd = nc.dram_tensor("wcv", [128, KT_C, CPC], f16, kind="ExternalInput").ap()
    wpd = nc.dram_tensor("wp", [CPC, C], f16, kind="ExternalInput").ap()
    bias6d = nc.dram_tensor("bias6", [CPC, 6], f32, kind="ExternalInput").ap()
    maskd = nc.dram_tensor("mask", [128, 128], f16, kind="ExternalInput").ap()
    outd = nc.dram_tensor("out", [NT, C], f16, kind="ExternalOutput").ap()
    import os
    DBG = bool(os.environ.get("KDBG"))
    if DBG:
        dqT = nc.dram_tensor("dqT", [128, NT], f16, kind="ExternalOutput").ap()
        dkT = nc.dram_tensor("dkT", [128, NT], f16, kind="ExternalOutput").ap()
        dqcT = nc.dram_tensor("dqcT", [128, NT], f16, kind="ExternalOutput").ap()
        dkcT = nc.dram_tensor("dkcT", [128, NTC], f16, kind="ExternalOutput").ap()
        dvn = nc.dram_tensor("dvn", [128, (NT // 128) * 2 * VW], f16, kind="ExternalOutput").ap()
        dyT2 = nc.dram_tensor("dyT2", [128, NT], f16, kind="ExternalOutput").ap()

    Exp = mybir.ActivationFunctionType.Exp
    DR = mybir.MatmulPerfMode.DoubleRow
    SCALE = 0.125  # 1/sqrt(D)

    with tile.TileContext(nc) as tc:
        from contextlib import ExitStack
        with ExitStack() as es:
            persist = es.enter_context(tc.tile_pool(name="persist", bufs=1))
            qT = persist.tile([128, NT], f16, tag="qT")
            kT = persist.tile([128, NT], f16, tag="kT")
            qcT = persist.tile([128, NT], f16, tag="qcT")
            kcT = persist.tile([128, NTC], f16, tag="kcT")
            vn = persist.tile([128, (NT // 128) * 2 * VW], f16, tag="vn")
            vcn = persist.tile([128, (NTC // 128) * 2 * VW], f16, tag="vcn")
            yT2 = persist.tile([128, NT], f16, tag="yT2")
            wp_t = persist.tile([128, C], f16, tag="wp")
            bias_t = persist.tile([128, 6], f32, tag="bias")
            mask_t = persist.tile([128, 128], f16, tag="mask")
            call16a = persist.tile([128, KT_C, 512], f16, tag="call16a")
            call16b = persist.tile([128, KT_C, 512], f16, tag="call16b")
            wq_t = persist.tile([128, KT_X, CPC], f16, tag="wqw")
            wk_t = persist.tile([128, KT_X, CPC], f16, tag="wkw")
            wcq_t = persist.tile([128, KT_X, CPC], f16, tag="wcqw")
            wck_t = persist.tile([128, KT_C, CPC], f16, tag="wckw")
            wv_t = persist.tile([128, KT_X, CPC], f16, tag="wvw")
            wcv_t = persist.tile([128, KT_C, CPC], f16, tag="wcvw")

            x16pool = es.enter_context(tc.tile_pool(name="x16pool", bufs=4))
            expool = es.enter_context(tc.tile_pool(name="expool", bufs=6))
            bpool = es.enter_context(tc.tile_pool(name="bpool", bufs=3))
            ypool = es.enter_context(tc.tile_pool(name="ypool", bufs=3))
            cpool = es.enter_context(tc.tile_pool(name="cpool", bufs=3))
            rdpool = es.enter_context(tc.tile_pool(name="rdpool", bufs=3, space="DRAM"))
            stps = es.enter_context(tc.tile_pool(name="stps", bufs=2, space="PSUM"))
            yh0ps = es.enter_context(tc.tile_pool(name="yh0ps", bufs=1, space="PSUM"))
            yh1ps = es.enter_context(tc.tile_pool(name="yh1ps", bufs=1, space="PSUM"))
            auxps = es.enter_context(tc.tile_pool(name="auxps", bufs=2, space="PSUM"))

            # rotating per-chunk x tiles; dict filled at DMA-issue time
            x16t = {}

            def issue_x16(ch, eng=None):
                if ch in x16t or ch >= NCH:
                    return
                t = x16pool.tile([128, KT_X, 512], f16, tag="x16c", name="x16tile")
                (eng or nc.sync).dma_start(out=t[:], in_=xTd16[ch])
                x16t[ch] = t

            # ---------------- input DMAs (multi-queue, critical first) ----
            # sync: score-path criticals (call16a whole — the SWDGE queue is
            # engine-driven and stalls behind gpsimd memsets)
            nc.sync.dma_start(out=wck_t[:], in_=wckd[:])
            nc.sync.dma_start(out=call16a[:, 0:2, :], in_=cTd16[0][:, 0:2, :])
            nc.sync.dma_start(out=call16a[:, 2:4, :], in_=cTd16[0][:, 2:4, :])
            nc.sync.dma_start(out=wcq_t[:], in_=wcqd[:])
            nc.sync.dma_start(out=wk_t[:], in_=wkd[:])
            nc.sync.dma_start(out=wq_t[:], in_=wqd[:])
            nc.sync.dma_start(out=mask_t[:], in_=maskd[:])
            nc.sync.dma_start(out=bias_t[:], in_=bias6d[:])
            # x16[0] split: front half on scalar, back half on gpsimd SWDGE
            # (issued before the memsets so the engine-driven queue flows)
            x0t = x16pool.tile([128, KT_X, 512], f16, tag="x16c", name="x16tile")
            nc.scalar.dma_start(out=x0t[:, 0:4, :], in_=xTd16[0][:, 0:4, :])
            nc.gpsimd.dma_start(out=x0t[:, 4:8, :], in_=xTd16[0][:, 4:8, :])
            x16t[0] = x0t
            nc.scalar.dma_start(out=wcv_t[:], in_=wcvd[:])
            nc.scalar.dma_start(out=wv_t[:], in_=wvd[:])
            issue_x16(1, nc.scalar)
            nc.scalar.dma_start(out=wp_t[:], in_=wpd[:])
            ones64 = persist.tile([1, 64], f16, tag="ones64")
            nc.vector.memset(ones64[:], 1.0)
            dpre = bpool.tile([1, 8], f32, tag="dpre")
            nc.vector.memset(dpre[:], 1.0)
            dpre2 = bpool.tile([1, 8], f16, tag="dpre2")
            nc.scalar.activation(dpre2[:], dpre[:], Exp, scale=1.0)
            nc.scalar.dma_start(out=call16b[:], in_=cTd16[1])
            vn_h = vn[:].rearrange("p (t h c) -> p t h c", h=2, c=VW)
            nc.gpsimd.memset(vn_h[:, :, :, 0:64], ALPHA)
            vcn_h = vcn[:].rearrange("p (t h c) -> p t h c", h=2, c=VW)
            nc.gpsimd.memset(vcn_h[:, :, :, 0:64], ALPHA)
            issue_x16(2, nc.scalar)

            # ---------------- filler machinery ----------------
            FILL = []            # list of (ns_est, fn, label_or_None)
            pending = set()

            def run_item(item):
                _, fn, label = item
                fn()
                if label is not None:
                    pending.discard(label)

            def pump(budget_ns):
                while budget_ns > 0 and FILL:
                    item = FILL.pop(0)
                    run_item(item)
                    budget_ns -= item[0]

            def require(label):
                while label in pending:
                    run_item(FILL.pop(0))

            def evict(dst, ps, bcol, eng):
                if zero_bias:
                    eng.tensor_copy(dst, ps)
                else:
                    eng.tensor_scalar_add(dst, ps, bias_t[:, bcol:bcol + 1])

            # fp16 channel-major projection: dst[:, ch*512:+512] = W^T x_chunk
            def proj_items(wtile, nkt, xsrc_fn, ch, dst, bcol, label):
                items = []
                state = {}
                for kt in range(nkt):
                    def f(kt=kt, state=state):
                        xsrc = xsrc_fn()
                        if kt == 0:
                            state['ps'] = auxps.tile([128, 512], f32, tag="aux", name="auxtile")
                        nc.tensor.matmul(state['ps'][:], wtile[:, kt, :],
                                         xsrc[:, kt, :],
                                         start=(kt == 0), stop=(kt == nkt - 1))
                        if kt == nkt - 1:
                            evict(dst[:, ch * 512:(ch + 1) * 512],
                                  state['ps'][:], bcol, nc.vector)
                    items.append((213, f, label if kt == nkt - 1 else None))
                return items

            # token-major V projection (fp16), one 128-token tile per group:
            # out[tok, ch] = sum_kt x[ckt, tok]^T @ Wv[ckt, ch], written into
            # the packed vn layout ([v64|ones] per head). V bias is added to
            # the normalized ya/yb in the part epilogue.
            def v_items(wtile, nkt, xsrc_fn, toff, dstn, dtile, label):
                items = []
                state = {}
                for kt in range(nkt):
                    def f(kt=kt, state=state):
                        xsrc = xsrc_fn()
                        if kt == 0:
                            state['ps'] = auxps.tile([128, 512], f32, tag="aux", name="auxtile")
                        nc.tensor.matmul(state['ps'][:, 0:128],
                                         xsrc[:, kt, toff * 128:(toff + 1) * 128],
                                         wtile[:, kt, :],
                                         start=(kt == 0), stop=(kt == nkt - 1))
                        if kt == nkt - 1:
                            ps = state['ps']
                            dr_ = dstn[:].rearrange(
                                "p (t h c) -> p t h c", h=2, c=VW)
                            nc.vector.tensor_copy(
                                dr_[:, dtile, :, 64:128],
                                ps[:, 0:128].rearrange("p (h c) -> p h c", h=2))
                    items.append((110, f, label if kt == nkt - 1 else None))
                return items

            def c_items(qlo, late=False):
                # out-projection for one 512-token chunk: 2-token-tile pairs,
                # one batched DMA per pair. Late mode (tail): PSUM from the
                # freed score pool, one [128,1024] eviction per token tile
                # alternating vector/scalar, out-DMA on the idle sync queue.
                items = []
                for tp in range(2):
                    tt0 = qlo // 128 + 2 * tp
                    state = {}
                    for ti in range(2):
                        for co in range(2):
                            def f(ti=ti, co=co, state=state, tt0=tt0, tp=tp):
                                if ti == 0 and co == 0:
                                    state['so'] = cpool.tile(
                                        [128, 2, C], f16, tag="so", name="sotile")
                                tt = tt0 + ti
                                if late:
                                    if co == 0:
                                        state['po'] = stps.tile(
                                            [128, 1024], f32, tag="st",
                                            name="sttile")
                                    po = state['po'][:, co * 512:(co + 1) * 512]
                                else:
                                    pot = auxps.tile([128, 512], f32, tag="aux",
                                                     name="auxtile")
                                    po = pot[:]
                                nc.tensor.matmul(po,
                                                 yT2[:, tt * 128:(tt + 1) * 128],
                                                 wp_t[:, co * 512:(co + 1) * 512],
                                                 start=True, stop=True)
                                if late:
                                    if co == 1:
                                        dst = state['so'][:, ti, :]
                                        if ti == 0:
                                            nc.vector.tensor_copy(
                                                dst, state['po'][:])
                                        else:
                                            nc.scalar.copy(dst, state['po'][:])
                                else:
                                    nc.vector.tensor_copy(
                                        state['so'][:, ti, co * 512:(co + 1) * 512],
                                        po)
                                if ti == 1 and co == 1:
                                    dst = outd[tt0 * 128:(tt0 + 2) * 128, :]
                                    dst = dst.rearrange("(t p) c -> p t c", t=2)
                                    deng = nc.sync if late else nc.gpsimd
                                    deng.dma_start(out=dst, in_=state['so'][:])
                            items.append((800, f, None))
                return items

            def add_group(items):
                if items[-1][2] is not None:
                    pending.add(items[-1][2])
                FILL.extend(items)

            c16s = lambda ch: (lambda: (call16a if ch == 0 else call16b))

            def x16f(ch):
                def g():
                    issue_x16(ch)
                    return x16t[ch]
                return g

            # ---------------- prologue: b0/qc0 projections ----------------
            for it in proj_items(wck_t, KT_C, c16s(0), 0, kcT, 4, None):
                it[1]()
            for it in proj_items(wcq_t, KT_X, x16f(0), 0, qcT, 3, None):
                it[1]()
            # everything else goes on the filler queue: the first scores need
            # only kcT + qcT; require-ahead drains vc0 before the first AV
            vi = []
            for tti in range(4):
                vi += v_items(wcv_t, KT_C, c16s(0), tti, vcn, tti,
                              "vc0" if tti == 3 else None)
            add_group(vi)
            add_group(proj_items(wk_t, KT_X, x16f(0), 0, kT, 1, "k0"))
            vi = []
            for tti in range(4):
                vi += v_items(wv_t, KT_X, x16f(0), tti, vn, tti,
                              "v0" if tti == 3 else None)
            add_group(vi)
            add_group(proj_items(wq_t, KT_X, x16f(0), 0, qT, 0, "q0"))

            # ---------------- filler queue: remaining projections ----------
            for ch in range(1, 4):
                add_group(proj_items(wcq_t, KT_X, x16f(ch), ch, qcT, 3, f"cq{ch}"))
                add_group(proj_items(wk_t, KT_X, x16f(ch), ch, kT, 1, f"k{ch}"))
                vi = []
                for tti in range(4):
                    vi += v_items(wv_t, KT_X, x16f(ch), tti, vn,
                                  ch * 4 + tti, f"v{ch}" if tti == 3 else None)
                add_group(vi)
                add_group(proj_items(wq_t, KT_X, x16f(ch), ch, qT, 0, f"q{ch}"))
            # cross b1
            add_group(proj_items(wck_t, KT_C, c16s(1), 1, kcT, 4, "kc1"))
            vi = []
            for tti in range(4):
                vi += v_items(wcv_t, KT_C, c16s(1), tti, vcn, 4 + tti,
                              "vc1" if tti == 3 else None)
            add_group(vi)
            B1_ORDER = [4, 5, 6, 7]
            for ch in B1_ORDER:
                add_group(proj_items(wcq_t, KT_X, x16f(ch), ch, qcT, 3, f"cq{ch}"))
                add_group(proj_items(wk_t, KT_X, x16f(ch), ch, kT, 1, f"k{ch}"))
                vi = []
                for tti in range(4):
                    vi += v_items(wv_t, KT_X, x16f(ch), tti, vn,
                                  ch * 4 + tti, f"v{ch}" if tti == 3 else None)
                add_group(vi)
                add_group(proj_items(wq_t, KT_X, x16f(ch), ch, qT, 0, f"q{ch}"))

            # ---------------- attention ----------------
            def attn_part(b, qc, qlo, is_self, last=False):
                nkt = (4 * qc + 4) if is_self else KT_C
                yh0 = yh0ps.tile([128, 512], f32, tag="yh0")
                yh1 = yh1ps.tile([128, 512], f32, tag="yh1")
                pend = []
                fidx = [0]

                def flush_one():
                    ex, off, vsrc, vc0, vc1 = pend.pop(0)
                    first = fidx[0] == 0
                    last = fidx[0] == nkt - 1
                    fidx[0] += 1
                    nc.tensor.matmul(yh0[:, off:512], vsrc[:, vc0:vc0 + VW],
                                     ex[:, off:512], start=first, stop=last)
                    nc.tensor.matmul(yh1[:, off:512], vsrc[:, vc1:vc1 + VW],
                                     ex[:, 512 + off:1024], start=first, stop=last)

                for kt in range(nkt):
                    if is_self:
                        crossing = kt >= 4 * qc
                        off = (kt - 4 * qc) * 128 if crossing else 0
                        klo = b * T + kt * 128
                        ksrc, qsrc, vsrc = kT, qT, vn
                        vbase = (b * KT_PER_B + kt) * 2 * VW
                    else:
                        crossing, off = False, 0
                        klo = b * TC + kt * 128
                        ksrc, qsrc, vsrc = kcT, qcT, vcn
                        vbase = (b * KT_C + kt) * 2 * VW
                    st = stps.tile([128, 1024], f32, tag="st")
                    nc.tensor.matmul(st[:, off:512],
                                     ksrc[0:64, klo:klo + 128],
                                     qsrc[0:64, qlo + off:qlo + 512],
                                     start=True, stop=True)
                    nc.tensor.matmul(st[:, 512 + off:1024],
                                     ksrc[64:128, klo:klo + 128],
                                     qsrc[64:128, qlo + off:qlo + 512],
                                     start=True, stop=True)
                    ex = expool.tile([128, 1024], f16, tag="ex")
                    if off == 0:
                        nc.scalar.activation(ex[:], st[:], Exp, scale=SCALE)
                    else:
                        st3 = st[:].rearrange("p (h q) -> p h q", h=2)[:, :, off:512]
                        ex3 = ex[:].rearrange("p (h q) -> p h q", h=2)[:, :, off:512]
                        nc.scalar.activation(ex3, st3, Exp, scale=SCALE)
                    if crossing:
                        # split across engines: both muls gate the AV flush
                        nc.vector.tensor_mul(ex[:, off:off + 128],
                                             ex[:, off:off + 128], mask_t[:])
                        nc.gpsimd.tensor_mul(ex[:, 512 + off:512 + off + 128],
                                             ex[:, 512 + off:512 + off + 128],
                                             mask_t[:])
                    pend.append((ex, off, vsrc, vbase, vbase + VW))
                    pump(430 if b == 0 else 600)
                    if len(pend) > 2:
                        flush_one()
                while pend:
                    pump(650)
                    flush_one()

                # epilogue: the V pad columns hold the ones-value, so PSUM
                # rows 0:64 carry ALPHA*den replicated across 64 partitions.
                # Reciprocal of [64,512] costs the same as [1,512] on DVE
                # (cost = free size), so no partition broadcast is needed:
                # ya = (ALPHA*y) * 1/(ALPHA*den) = y/den directly.
                rs0 = bpool.tile([64, 512], f32, tag="rs0")
                nc.vector.reciprocal_approx_fast(rs0[:], yh0[0:64, :])
                rs1 = bpool.tile([64, 512], f32, tag="rs1")
                nc.vector.reciprocal_approx_fast(rs1[:], yh1[0:64, :])
                ysb = bpool.tile([64, 1024], f16, tag="ysb")
                nc.vector.tensor_scalar_mul(ysb[:, 0:512], yh0[64:128, :], ALPHA)
                # scalar's exp queue is drained at part end: split PSUM reads
                nc.scalar.activation(ysb[:, 512:1024], yh1[64:128, :],
                                     mybir.ActivationFunctionType.Copy,
                                     scale=ALPHA)
                ya = ypool.tile([64, 512], f16, tag="ya")
                yb = ypool.tile([64, 512], f16, tag="yb")
                nc.vector.tensor_mul(ya[:], ysb[:, 0:512], rs0[:])
                nc.gpsimd.tensor_mul(yb[:], ysb[:, 512:1024], rs1[:])
                if not zero_bias:
                    vb = 2 if is_self else 5
                    nc.gpsimd.tensor_scalar_add(ya[:], ya[:],
                                                bias_t[0:64, vb:vb + 1])
                    nc.gpsimd.tensor_scalar_add(yb[:], yb[:],
                                                bias_t[64:128, vb:vb + 1])
                pump(1200 if b == 0 else 1500)
                return ya, yb

            PROC = [0, 1, 2, 3] + B1_ORDER
            for step, chk in enumerate(PROC):
                    b, qc = chk // 4, chk % 4
                    qlo = b * T + qc * 512
                    # stream in x chunks two parts ahead (processing order)
                    if step + 2 < len(PROC):
                        issue_x16(PROC[step + 2])
                    if b == 1:
                        require("kc1")
                        require("vc1")
                    if chk > 0:
                        require(f"cq{chk}")
                    else:
                        require("vc0")
                    # hoist: finish this chunk's K/V/Q before the cross part
                    # so their evictions land well ahead of the self scores
                    require(f"k{chk}")
                    require(f"v{chk}")
                    require(f"q{chk}")
                    ya_c, yb_c = attn_part(b, qc, qlo, is_self=False)
                    # prefetch the next chunk's CQ projection (its eviction
                    # then completes during this self part, not at the
                    # boundary right before the next cross scores need it)
                    if step + 1 < len(PROC):
                        require(f"cq{PROC[step + 1]}")
                    ya_s, yb_s = attn_part(b, qc, qlo, is_self=True,
                                           last=(step == len(PROC) - 1))
                    nc.vector.tensor_add(yT2[0:64, qlo:qlo + 512],
                                         ya_s[:], ya_c[:])
                    nc.gpsimd.tensor_add(yT2[64:128, qlo:qlo + 512],
                                         yb_s[:], yb_c[:])
                    add_group(c_items(qlo, late=(step == len(PROC) - 1)))

            while FILL:
                run_item(FILL.pop(0))

            if DBG:
                nc.sync.dma_start(out=dqT[:], in_=qT[:])
                nc.sync.dma_start(out=dkT[:], in_=kT[:])
                nc.sync.dma_start(out=dqcT[:], in_=qcT[:])
                nc.sync.dma_start(out=dkcT[:], in_=kcT[:])
                nc.sync.dma_start(out=dvn[:], in_=vn[:])
                nc.sync.dma_start(out=dyT2[:], in_=yT2[:])

    nc.compile()
    return nc


_NC_CACHE = {}


def _get_nc(zero_bias=False):
    if zero_bias not in _NC_CACHE:
        _NC_CACHE[zero_bias] = _build(zero_bias)
    return _NC_CACHE[zero_bias]


F8 = ml_dtypes.float8_e4m3


def warr(w, dtype):
    """[C,128] weight -> [128, KT, 128] (partition-major k-tiles)."""
    kt = w.shape[0] // 128
    return np.ascontiguousarray(
        w.reshape(kt, 128, w.shape[1]).transpose(1, 0, 2)).astype(dtype)


def _xt(x, nt, ktx, dtype):
    """[nt, c] -> [nt/512, 128, ktx, 512]."""
    xT0 = x.T.astype(dtype)  # [c, nt]
    return np.ascontiguousarray(
        xT0.reshape(ktx, 128, nt // 512, 512).transpose(2, 1, 0, 3))


def make_in_maps(x, cross_input, Wk, bk, Wq, bq, Wv, bv, Wck, bck, Wcq, bcq,
                 Wcv, bcv, Wp, bp):
    """Host-side shard + layout prep. Returns per-core input maps."""
    x2 = np.asarray(x, np.float32).reshape(NT, C)
    c2 = np.asarray(cross_input, np.float32).reshape(NTC, CC)
    xT16 = _xt(x2, NT, KT_X, np.float16)
    cT16 = _xt(c2, NTC, KT_C, np.float16)
    mask = np.triu(np.ones((128, 128), np.float32)).astype(np.float16)  # 1 iff kk<=qq
    Wq, Wk, Wv = (np.asarray(w, np.float32) for w in (Wq, Wk, Wv))
    Wcq, Wck, Wcv = (np.asarray(w, np.float32) for w in (Wcq, Wck, Wcv))
    Wp = np.asarray(Wp, np.float32)
    in_maps = []
    for c in range(NCORES):
        sl = slice(c * CPC, (c + 1) * CPC)
        bias6 = np.stack([np.asarray(v, np.float32)[sl] for v in
                          (bq, bk, bv, bcq, bck, bcv)], axis=1)
        in_maps.append({
            "xT16": xT16, "cT16": cT16,
            "wq": warr(Wq[:, sl], np.float16), "wk": warr(Wk[:, sl], np.float16),
            "wcq": warr(Wcq[:, sl], np.float16), "wck": warr(Wck[:, sl], np.float16),
            "wv": warr(Wv[:, sl], np.float16),
            "wcv": warr(Wcv[:, sl], np.float16),
            "wp": Wp[sl, :].astype(np.float16),
            "bias6": np.ascontiguousarray(bias6),
            "mask": mask,
        })
    return in_maps


def kernel(**inputs):
    in_maps = make_in_maps(**inputs)
    zb = all(not np.any(np.asarray(inputs[k])) for k in
             ("bq", "bk", "bv", "bcq", "bck", "bcv"))
    nc = _get_nc(zero_bias=zb)
    res = run_bass_kernel_spmd(nc, in_maps, list(range(NCORES)))
    acc = np.zeros((NT, C), np.float64)
    for c in range(NCORES):
        acc += res.results[c]["out"]
    acc += np.asarray(inputs["bp"], np.float32)
    return acc.reshape(B, T, C).astype(np.float32)


if __name__ == "__main__":
    nc = _get_nc()
    print("build + compile OK")


# revision 44
# speedup vs baseline: 1.0338x; 1.0338x over previous
"""Causal self-attention + cross-attention Trainium2 kernel (8 NeuronCores).

Sharding: head-parallel. 16 heads x 2 batches = 32 (b,h) pairs; core c owns
heads {2c, 2c+1} for both batches (its 128 channels of C=1024). Projections
are column-sliced per core; attention runs fully local per head; the output
projection is row-sliced and the 8 partial [NT, C] fp32 outputs are summed
on the host (no device collectives).

v3 (fp16 everywhere; fp8 was tried and rejected — softmax reweighting
noise from quantized Q/K lands ~5% relative error on y, over the 2e-2
gate): same software-pipelined emission as v2, with schedule fixes found
from NTFF traces:
- x chunks stream through a rotating 4-buffer SBUF pool; input DMAs are
  spread across the sync/scalar/gpsimd queues (each queue sustains only
  ~120 GB/s) with the first-needed tensors on their own queues, and
  cross-attention inputs split per batch so no consumer waits on a late
  whole-tile write.
- fillers are pumped BEFORE each AV flush (the PE queue is in-order; a
  flush waiting on exp would block fillers emitted after it).
- the softmax epilogue runs recips first (they gate the broadcast), reads
  yh head-b rows on the scalar engine (drained of exps at part end), puts
  the ya path on vector and yb on gpsimd in parallel, and writes head-b
  rows straight into yT2[64:128] (no SBUF shift DMA).
- out-projection items are spread across the following part as boundary
  filler; their output DMAs are 2-token-tile batched on the SWDGE queue.
- the V pad columns hold the ones-value, so the AV matmul replicates
  ALPHA*den across PSUM partitions 0:64 for free; the reciprocal runs on
  [64,512] at the same DVE cost as [1,512] (cost = free size), removing
  the per-part DRAM-roundtrip partition broadcast entirely.
- the final chunk's out-projection uses the freed score PSUM banks with
  vector+scalar evictions, and its output DMAs go on the idle sync queue.
"""
import sys

sys.path.insert(0, "/opt/trn_rl_repo")

import numpy as np
import ml_dtypes

import concourse.bass as bass
import concourse.tile as tile
from concourse import bacc, mybir
from concourse.bass_utils import run_bass_kernel_spmd

dt = mybir.dt

B, T, TC, C, CC, H, D = 2, 2048, 512, 1024, 512, 16, 64
NCORES = 8
CPC = 128          # channels per core = 2 heads * 64
NT = B * T         # 4096 tokens (batch-major)
NTC = B * TC       # 1024 cross tokens
KT_X = C // 128    # 8 contraction tiles over C
KT_C = CC // 128   # 4 contraction tiles over CC
NCH = NT // 512    # 8 token chunks (b0: 0-3, b1: 4-7)
NCHC = NTC // 512  # 2 cross chunks (b0, b1)
QC_PER_B = T // 512
KT_PER_B = T // 128
VW = 128           # V block cols per head: [ones | pad63 | v64]
ALPHA = 1.0 / 64   # ones-column value; denominators come out scaled by ALPHA


def _build(zero_bias=False):
    f32, f16, f8 = dt.float32, dt.float16, dt.float8e4
    nc = bacc.Bacc("TRN2", target_bir_lowering=False, debug=False,
                   enable_asserts=True, num_devices=NCORES)

    xTd16 = nc.dram_tensor("xT16", [NCH, 128, KT_X, 512], f16, kind="ExternalInput").ap()
    cTd16 = nc.dram_tensor("cT16", [NCHC, 128, KT_C, 512], f16, kind="ExternalInput").ap()
    wqd = nc.dram_tensor("wq", [128, KT_X, CPC], f16, kind="ExternalInput").ap()
    wkd = nc.dram_tensor("wk", [128, KT_X, CPC], f16, kind="ExternalInput").ap()
    wcqd = nc.dram_tensor("wcq", [128, KT_X, CPC], f16, kind="ExternalInput").ap()
    wckd = nc.dram_tensor("wck", [128, KT_C, CPC], f16, kind="ExternalInput").ap()
    wvd = nc.dram_tensor("wv", [128, KT_X, CPC], f16, kind="ExternalInput").ap()
    wcvd = nc.dram_tensor("wcv", [128, KT_C, CPC], f16, kind="ExternalInput").ap()
    wpd = nc.dram_tensor("wp", [CPC, C], f16, kind="ExternalInput").ap()
    bias6d = nc.dram_tensor("bias6", [CPC, 6], f32, kind="ExternalInput").ap()
    maskd = nc.dram_tensor("mask", [128, 128], f16, kind="ExternalInput").ap()
    outd = nc.dram_tensor("out", [NT, C], f16, kind="ExternalOutput").ap()
    import os
    DBG = bool(os.environ.get("KDBG"))
    if DBG:
        dqT = nc.dram_tensor("dqT", [128, NT], f16, kind="ExternalOutput").ap()
        dkT = nc.dram_tensor("dkT", [128, NT], f16, kind="ExternalOutput").ap()
        dqcT = nc.dram_tensor("dqcT", [128, NT], f16, kind="ExternalOutput").ap()
        dkcT = nc.dram_tensor("dkcT", [128, NTC], f16, kind="ExternalOutput").ap()
        dvn = nc.dram_tensor("dvn", [128, (NT // 128) * 2 * VW], f16, kind="ExternalOutput").ap()
        dyT2 = nc.dram_tensor("dyT2", [128, NT], f16, kind="ExternalOutput").ap()

    Exp = mybir.ActivationFunctionType.Exp
    DR = mybir.MatmulPerfMode.DoubleRow
    SCALE = 0.125  # 1/sqrt(D)

    with tile.TileContext(nc) as tc:
        from contextlib import ExitStack
        with ExitStack() as es:
            persist = es.enter_context(tc.tile_pool(name="persist", bufs=1))
            qT = persist.tile([128, NT], f16, tag="qT")
            kT = persist.tile([128, NT], f16, tag="kT")
            qcT = persist.tile([128, NT], f16, tag="qcT")
            kcT = persist.tile([128, NTC], f16, tag="kcT")
            vn = persist.tile([128, (NT // 128) * 2 * VW], f16, tag="vn")
            vcn = persist.tile([128, (NTC // 128) * 2 * VW], f16, tag="vcn")
            yT2 = persist.tile([128, NT], f16, tag="yT2")
            wp_t = persist.tile([128, C], f16, tag="wp")
            bias_t = persist.tile([128, 6], f32, tag="bias")
            mask_t = persist.tile([128, 128], f16, tag="mask")
            call16a = persist.tile([128, KT_C, 512], f16, tag="call16a")
            call16b = persist.tile([128, KT_C, 512], f16, tag="call16b")
            wq_t = persist.tile([128, KT_X, CPC], f16, tag="wqw")
            wk_t = persist.tile([128, KT_X, CPC], f16, tag="wkw")
            wcq_t = persist.tile([128, KT_X, CPC], f16, tag="wcqw")
            wck_t = persist.tile([128, KT_C, CPC], f16, tag="wckw")
            wv_t = persist.tile([128, KT_X, CPC], f16, tag="wvw")
            wcv_t = persist.tile([128, KT_C, CPC], f16, tag="wcvw")

            x16pool = es.enter_context(tc.tile_pool(name="x16pool", bufs=4))
            expool = es.enter_context(tc.tile_pool(name="expool", bufs=6))
            bpool = es.enter_context(tc.tile_pool(name="bpool", bufs=3))
            ypool = es.enter_context(tc.tile_pool(name="ypool", bufs=3))
            cpool = es.enter_context(tc.tile_pool(name="cpool", bufs=3))
            rdpool = es.enter_context(tc.tile_pool(name="rdpool", bufs=3, space="DRAM"))
            stps = es.enter_context(tc.tile_pool(name="stps", bufs=2, space="PSUM"))
            yh0ps = es.enter_context(tc.tile_pool(name="yh0ps", bufs=1, space="PSUM"))
            yh1ps = es.enter_context(tc.tile_pool(name="yh1ps", bufs=1, space="PSUM"))
            auxps = es.enter_context(tc.tile_pool(name="auxps", bufs=2, space="PSUM"))

            # rotating per-chunk x tiles; dict filled at DMA-issue time
            x16t = {}

            def issue_x16(ch, eng=None):
                if ch in x16t or ch >= NCH:
                    return
                t = x16pool.tile([128, KT_X, 512], f16, tag="x16c", name="x16tile")
                (eng or nc.sync).dma_start(out=t[:], in_=xTd16[ch])
                x16t[ch] = t

            # ---------------- input DMAs (multi-queue, critical first) ----
            # sync: score-path criticals (call16a whole — the SWDGE queue is
            # engine-driven and stalls behind gpsimd memsets)
            nc.sync.dma_start(out=wck_t[:], in_=wckd[:])
            nc.sync.dma_start(out=call16a[:, 0:2, :], in_=cTd16[0][:, 0:2, :])
            nc.sync.dma_start(out=call16a[:, 2:4, :], in_=cTd16[0][:, 2:4, :])
            nc.sync.dma_start(out=wcq_t[:], in_=wcqd[:])
            nc.sync.dma_start(out=wk_t[:], in_=wkd[:])
            nc.sync.dma_start(out=wq_t[:], in_=wqd[:])
            nc.sync.dma_start(out=mask_t[:], in_=maskd[:])
            nc.sync.dma_start(out=bias_t[:], in_=bias6d[:])
            # x16[0] split: front half on scalar, back half on gpsimd SWDGE
            # (issued before the memsets so the engine-driven queue flows)
            x0t = x16pool.tile([128, KT_X, 512], f16, tag="x16c", name="x16tile")
            nc.scalar.dma_start(out=x0t[:, 0:4, :], in_=xTd16[0][:, 0:4, :])
            nc.gpsimd.dma_start(out=x0t[:, 4:8, :], in_=xTd16[0][:, 4:8, :])
            x16t[0] = x0t
            nc.scalar.dma_start(out=wcv_t[:], in_=wcvd[:])
            nc.scalar.dma_start(out=wv_t[:], in_=wvd[:])
            issue_x16(1, nc.scalar)
            nc.scalar.dma_start(out=wp_t[:], in_=wpd[:])
            ones64 = persist.tile([1, 64], f16, tag="ones64")
            nc.vector.memset(ones64[:], 1.0)
            dpre = bpool.tile([1, 8], f32, tag="dpre")
            nc.vector.memset(dpre[:], 1.0)
            dpre2 = bpool.tile([1, 8], f16, tag="dpre2")
            nc.scalar.activation(dpre2[:], dpre[:], Exp, scale=1.0)
            nc.scalar.dma_start(out=call16b[:], in_=cTd16[1])
            vn_h = vn[:].rearrange("p (t h c) -> p t h c", h=2, c=VW)
            nc.gpsimd.memset(vn_h[:, :, :, 0:64], ALPHA)
            vcn_h = vcn[:].rearrange("p (t h c) -> p t h c", h=2, c=VW)
            nc.gpsimd.memset(vcn_h[:, :, :, 0:64], ALPHA)
            issue_x16(2, nc.scalar)

            # ---------------- filler machinery ----------------
            FILL = []            # list of (ns_est, fn, label_or_None)
            pending = set()

            def run_item(item):
                _, fn, label = item
                fn()
                if label is not None:
                    pending.discard(label)

            def pump(budget_ns):
                while budget_ns > 0 and FILL:
                    item = FILL.pop(0)
                    run_item(item)
                    budget_ns -= item[0]

            def require(label):
                while label in pending:
                    run_item(FILL.pop(0))

            def evict(dst, ps, bcol, eng):
                if zero_bias:
                    eng.tensor_copy(dst, ps)
                else:
                    eng.tensor_scalar_add(dst, ps, bias_t[:, bcol:bcol + 1])

            # fp16 channel-major projection: dst[:, ch*512:+512] = W^T x_chunk
            def proj_items(wtile, nkt, xsrc_fn, ch, dst, bcol, label):
                items = []
                state = {}
                for kt in range(nkt):
                    def f(kt=kt, state=state):
                        xsrc = xsrc_fn()
                        if kt == 0:
                            state['ps'] = auxps.tile([128, 512], f32, tag="aux", name="auxtile")
                        nc.tensor.matmul(state['ps'][:], wtile[:, kt, :],
                                         xsrc[:, kt, :],
                                         start=(kt == 0), stop=(kt == nkt - 1))
                        if kt == nkt - 1:
                            evict(dst[:, ch * 512:(ch + 1) * 512],
                                  state['ps'][:], bcol, nc.vector)
                    items.append((213, f, label if kt == nkt - 1 else None))
                return items

            # token-major V projection (fp16), one 128-token tile per group:
            # out[tok, ch] = sum_kt x[ckt, tok]^T @ Wv[ckt, ch], written into
            # the packed vn layout ([v64|ones] per head). V bias is added to
            # the normalized ya/yb in the part epilogue.
            def v_items(wtile, nkt, xsrc_fn, toff, dstn, dtile, label):
                items = []
                state = {}
                for kt in range(nkt):
                    def f(kt=kt, state=state):
                        xsrc = xsrc_fn()
                        if kt == 0:
                            state['ps'] = auxps.tile([128, 512], f32, tag="aux", name="auxtile")
                        nc.tensor.matmul(state['ps'][:, 0:128],
                                         xsrc[:, kt, toff * 128:(toff + 1) * 128],
                                         wtile[:, kt, :],
                                         start=(kt == 0), stop=(kt == nkt - 1))
                        if kt == nkt - 1:
                            ps = state['ps']
                            dr_ = dstn[:].rearrange(
                                "p (t h c) -> p t h c", h=2, c=VW)
                            nc.vector.tensor_copy(
                                dr_[:, dtile, :, 64:128],
                                ps[:, 0:128].rearrange("p (h c) -> p h c", h=2))
                    items.append((110, f, label if kt == nkt - 1 else None))
                return items

            def c_items(qlo, late=False):
                # out-projection for one 512-token chunk: 2-token-tile pairs,
                # one batched DMA per pair. Late mode (tail): PSUM from the
                # freed score pool, one [128,1024] eviction per token tile
                # alternating vector/scalar, out-DMA on the idle sync queue.
                items = []
                for tp in range(2):
                    tt0 = qlo // 128 + 2 * tp
                    state = {}
                    for ti in range(2):
                        for co in range(2):
                            def f(ti=ti, co=co, state=state, tt0=tt0, tp=tp):
                                if ti == 0 and co == 0:
                                    state['so'] = cpool.tile(
                                        [128, 2, C], f16, tag="so", name="sotile")
                                tt = tt0 + ti
                                if late:
                                    if co == 0:
                                        state['po'] = stps.tile(
                                            [128, 1024], f32, tag="st",
                                            name="sttile")
                                    po = state['po'][:, co * 512:(co + 1) * 512]
                                else:
                                    pot = auxps.tile([128, 512], f32, tag="aux",
                                                     name="auxtile")
                                    po = pot[:]
                                nc.tensor.matmul(po,
                                                 yT2[:, tt * 128:(tt + 1) * 128],
                                                 wp_t[:, co * 512:(co + 1) * 512],
                                                 start=True, stop=True)
                                if late:
                                    if co == 1:
                                        dst = state['so'][:, ti, :]
                                        if ti == 0:
                                            nc.vector.tensor_copy(
                                                dst, state['po'][:])
                                        else:
                                            nc.scalar.copy(dst, state['po'][:])
                                else:
                                    nc.vector.tensor_copy(
                                        state['so'][:, ti, co * 512:(co + 1) * 512],
                                        po)
                                if ti == 1 and co == 1:
                                    dst = outd[tt0 * 128:(tt0 + 2) * 128, :]
                                    dst = dst.rearrange("(t p) c -> p t c", t=2)
                                    deng = nc.sync if late else nc.gpsimd
                                    deng.dma_start(out=dst, in_=state['so'][:])
                            items.append((800, f, None))
                return items

            def add_group(items):
                if items[-1][2] is not None:
                    pending.add(items[-1][2])
                FILL.extend(items)

            c16s = lambda ch: (lambda: (call16a if ch == 0 else call16b))

            def x16f(ch):
                def g():
                    issue_x16(ch)
                    return x16t[ch]
                return g

            # ---------------- prologue: b0/qc0 projections ----------------
            for it in proj_items(wck_t, KT_C, c16s(0), 0, kcT, 4, None):
                it[1]()
            for it in proj_items(wcq_t, KT_X, x16f(0), 0, qcT, 3, None):
                it[1]()
            # everything else goes on the filler queue: the first scores need
            # only kcT + qcT; require-ahead drains vc0 before the first AV
            vi = []
            for tti in range(4):
                vi += v_items(wcv_t, KT_C, c16s(0), tti, vcn, tti,
                              "vc0" if tti == 3 else None)
            add_group(vi)
            add_group(proj_items(wk_t, KT_X, x16f(0), 0, kT, 1, "k0"))
            vi = []
            for tti in range(4):
                vi += v_items(wv_t, KT_X, x16f(0), tti, vn, tti,
                              "v0" if tti == 3 else None)
            add_group(vi)
            add_group(proj_items(wq_t, KT_X, x16f(0), 0, qT, 0, "q0"))

            # ---------------- filler queue: remaining projections ----------
            for ch in range(1, 4):
                add_group(proj_items(wcq_t, KT_X, x16f(ch), ch, qcT, 3, f"cq{ch}"))
                add_group(proj_items(wk_t, KT_X, x16f(ch), ch, kT, 1, f"k{ch}"))
                vi = []
                for tti in range(4):
                    vi += v_items(wv_t, KT_X, x16f(ch), tti, vn,
                                  ch * 4 + tti, f"v{ch}" if tti == 3 else None)
                add_group(vi)
                add_group(proj_items(wq_t, KT_X, x16f(ch), ch, qT, 0, f"q{ch}"))
            # cross b1
            add_group(proj_items(wck_t, KT_C, c16s(1), 1, kcT, 4, "kc1"))
            vi = []
            for tti in range(4):
                vi += v_items(wcv_t, KT_C, c16s(1), tti, vcn, 4 + tti,
                              "vc1" if tti == 3 else None)
            add_group(vi)
            B1_ORDER = [4, 5, 6, 7]
            for ch in B1_ORDER:
                add_group(proj_items(wcq_t, KT_X, x16f(ch), ch, qcT, 3, f"cq{ch}"))
                add_group(proj_items(wk_t, KT_X, x16f(ch), ch, kT, 1, f"k{ch}"))
                vi = []
                for tti in range(4):
                    vi += v_items(wv_t, KT_X, x16f(ch), tti, vn,
                                  ch * 4 + tti, f"v{ch}" if tti == 3 else None)
                add_group(vi)
                add_group(proj_items(wq_t, KT_X, x16f(ch), ch, qT, 0, f"q{ch}"))

            # ---------------- attention ----------------
            def attn_part(b, qc, qlo, is_self, last=False):
                nkt = (4 * qc + 4) if is_self else KT_C
                yh0 = yh0ps.tile([128, 512], f32, tag="yh0")
                yh1 = yh1ps.tile([128, 512], f32, tag="yh1")
                pend = []
                fidx = [0]

                def flush_one():
                    ex, off, vsrc, vc0, vc1 = pend.pop(0)
                    first = fidx[0] == 0
                    last = fidx[0] == nkt - 1
                    fidx[0] += 1
                    nc.tensor.matmul(yh0[:, off:512], vsrc[:, vc0:vc0 + VW],
                                     ex[:, off:512], start=first, stop=last)
                    nc.tensor.matmul(yh1[:, off:512], vsrc[:, vc1:vc1 + VW],
                                     ex[:, 512 + off:1024], start=first, stop=last)

                for kt in range(nkt):
                    if is_self:
                        crossing = kt >= 4 * qc
                        off = (kt - 4 * qc) * 128 if crossing else 0
                        klo = b * T + kt * 128
                        ksrc, qsrc, vsrc = kT, qT, vn
                        vbase = (b * KT_PER_B + kt) * 2 * VW
                    else:
                        crossing, off = False, 0
                        klo = b * TC + kt * 128
                        ksrc, qsrc, vsrc = kcT, qcT, vcn
                        vbase = (b * KT_C + kt) * 2 * VW
                    st = stps.tile([128, 1024], f32, tag="st")
                    nc.tensor.matmul(st[:, off:512],
                                     ksrc[0:64, klo:klo + 128],
                                     qsrc[0:64, qlo + off:qlo + 512],
                                     start=True, stop=True)
                    nc.tensor.matmul(st[:, 512 + off:1024],
                                     ksrc[64:128, klo:klo + 128],
                                     qsrc[64:128, qlo + off:qlo + 512],
                                     start=True, stop=True)
                    ex = expool.tile([128, 1024], f16, tag="ex")
                    if off == 0:
                        nc.scalar.activation(ex[:], st[:], Exp, scale=SCALE)
                    else:
                        st3 = st[:].rearrange("p (h q) -> p h q", h=2)[:, :, off:512]
                        ex3 = ex[:].rearrange("p (h q) -> p h q", h=2)[:, :, off:512]
                        nc.scalar.activation(ex3, st3, Exp, scale=SCALE)
                    if crossing:
                        # split across engines: both muls gate the AV flush
                        nc.vector.tensor_mul(ex[:, off:off + 128],
                                             ex[:, off:off + 128], mask_t[:])
                        nc.gpsimd.tensor_mul(ex[:, 512 + off:512 + off + 128],
                                             ex[:, 512 + off:512 + off + 128],
                                             mask_t[:])
                    pend.append((ex, off, vsrc, vbase, vbase + VW))
                    pump(430 if b == 0 else 600)
                    if len(pend) > 2:
                        flush_one()
                while pend:
                    pump(650)
                    flush_one()

                # epilogue: the V pad columns hold the ones-value, so PSUM
                # rows 0:64 carry ALPHA*den replicated across 64 partitions.
                # Reciprocal of [64,512] costs the same as [1,512] on DVE
                # (cost = free size), so no partition broadcast is needed:
                # ya = (ALPHA*y) * 1/(ALPHA*den) = y/den directly.
                rs0 = bpool.tile([64, 512], f32, tag="rs0")
                nc.vector.reciprocal_approx_fast(rs0[:], yh0[0:64, :])
                rs1 = bpool.tile([64, 512], f32, tag="rs1")
                nc.vector.reciprocal_approx_fast(rs1[:], yh1[0:64, :])
                ysb = bpool.tile([64, 1024], f16, tag="ysb")
                nc.vector.tensor_scalar_mul(ysb[:, 0:512], yh0[64:128, :], ALPHA)
                # scalar's exp queue is drained at part end: split PSUM reads
                nc.scalar.activation(ysb[:, 512:1024], yh1[64:128, :],
                                     mybir.ActivationFunctionType.Copy,
                                     scale=ALPHA)
                ya = ypool.tile([64, 512], f16, tag="ya")
                yb = ypool.tile([64, 512], f16, tag="yb")
                nc.vector.tensor_mul(ya[:], ysb[:, 0:512], rs0[:])
                nc.gpsimd.tensor_mul(yb[:], ysb[:, 512:1024], rs1[:])
                if not zero_bias:
                    vb = 2 if is_self else 5
                    nc.gpsimd.tensor_scalar_add(ya[:], ya[:],
                                                bias_t[0:64, vb:vb + 1])
                    nc.gpsimd.tensor_scalar_add(yb[:], yb[:],
                                                bias_t[64:128, vb:vb + 1])
                pump(1200 if b == 0 else 1500)
                return ya, yb

            PROC = [0, 1, 2, 3] + B1_ORDER
            for step, chk in enumerate(PROC):
                    b, qc = chk // 4, chk % 4
                    qlo = b * T + qc * 512
                    # stream in x chunks two parts ahead (processing order)
                    if step + 2 < len(PROC):
                        issue_x16(PROC[step + 2])
                    if b == 1:
                        require("kc1")
                        require("vc1")
                    if chk > 0:
                        require(f"cq{chk}")
                    else:
                        require("vc0")
                    ya_c, yb_c = attn_part(b, qc, qlo, is_self=False)
                    require(f"k{chk}")
                    require(f"v{chk}")
                    require(f"q{chk}")
                    ya_s, yb_s = attn_part(b, qc, qlo, is_self=True,
                                           last=(step == len(PROC) - 1))
                    nc.vector.tensor_add(yT2[0:64, qlo:qlo + 512],
                                         ya_s[:], ya_c[:])
                    nc.gpsimd.tensor_add(yT2[64:128, qlo:qlo + 512],
                                         yb_s[:], yb_c[:])
                    add_group(c_items(qlo, late=(step == len(PROC) - 1)))

            while FILL:
                run_item(FILL.pop(0))

            if DBG:
                nc.sync.dma_start(out=dqT[:], in_=qT[:])
                nc.sync.dma_start(out=dkT[:], in_=kT[:])
                nc.sync.dma_start(out=dqcT[:], in_=qcT[:])
                nc.sync.dma_start(out=dkcT[:], in_=kcT[:])
                nc.sync.dma_start(out=dvn[:], in_=vn[:])
                nc.sync.dma_start(out=dyT2[:], in_=yT2[:])

    nc.compile()
    return nc


_NC_CACHE = {}


def _get_nc(zero_bias=False):
    if zero_bias not in _NC_CACHE:
        _NC_CACHE[zero_bias] = _build(zero_bias)
    return _NC_CACHE[zero_bias]


F8 = ml_dtypes.float8_e4m3


def warr(w, dtype):
    """[C,128] weight -> [128, KT, 128] (partition-major k-tiles)."""
    kt = w.shape[0] // 128
    return np.ascontiguousarray(
        w.reshape(kt, 128, w.shape[1]).transpose(1, 0, 2)).astype(dtype)


def _xt(x, nt, ktx, dtype):
    """[nt, c] -> [nt/512, 128, ktx, 512]."""
    xT0 = x.T.astype(dtype)  # [c, nt]
    return np.ascontiguousarray(
        xT0.reshape(ktx, 128, nt // 512, 512).transpose(2, 1, 0, 3))


def make_in_maps(x, cross_input, Wk, bk, Wq, bq, Wv, bv, Wck, bck, Wcq, bcq,
                 Wcv, bcv, Wp, bp):
    """Host-side shard + layout prep. Returns per-core input maps."""
    x2 = np.asarray(x, np.float32).reshape(NT, C)
    c2 = np.asarray(cross_input, np.float32).reshape(NTC, CC)
    xT16 = _xt(x2, NT, KT_X, np.float16)
    cT16 = _xt(c2, NTC, KT_C, np.float16)
    mask = np.triu(np.ones((128, 128), np.float32)).astype(np.float16)  # 1 iff kk<=qq
    Wq, Wk, Wv = (np.asarray(w, np.float32) for w in (Wq, Wk, Wv))
    Wcq, Wck, Wcv = (np.asarray(w, np.float32) for w in (Wcq, Wck, Wcv))
    Wp = np.asarray(Wp, np.float32)
    in_maps = []
    for c in range(NCORES):
        sl = slice(c * CPC, (c + 1) * CPC)
        bias6 = np.stack([np.asarray(v, np.float32)[sl] for v in
                          (bq, bk, bv, bcq, bck, bcv)], axis=1)
        in_maps.append({
            "xT16": xT16, "cT16": cT16,
            "wq": warr(Wq[:, sl], np.float16), "wk": warr(Wk[:, sl], np.float16),
            "wcq": warr(Wcq[:, sl], np.float16), "wck": warr(Wck[:, sl], np.float16),
            "wv": warr(Wv[:, sl], np.float16),
            "wcv": warr(Wcv[:, sl], np.float16),
            "wp": Wp[sl, :].astype(np.float16),
            "bias6": np.ascontiguousarray(bias6),
            "mask": mask,
        })
    return in_maps


def kernel(**inputs):
    in_maps = make_in_maps(**inputs)
    zb = all(not np.any(np.asarray(inputs[k])) for k in
             ("bq", "bk", "bv", "bcq", "bck", "bcv"))
    nc = _get_nc(zero_bias=zb)
    res = run_bass_kernel_spmd(nc, in_maps, list(range(NCORES)))
    acc = np.zeros((NT, C), np.float64)
    for c in range(NCORES):
        acc += res.results[c]["out"]
    acc += np.asarray(inputs["bp"], np.float32)
    return acc.reshape(B, T, C).astype(np.float32)


if __name__ == "__main__":
    nc = _get_nc()
    print("build + compile OK")
